# revision 93
# baseline (speedup 1.0000x reference)
"""Trainium2 Bass kernel for the nn_Points problem.

Renders N=1024 anisotropic "diamond" points onto a 3x256x384 canvas:
    t = (pixel - loc) @ M_n          (2-vector per pixel per point)
    mapped = relu(1 - (|t0|+|t1|)/2)
    canvas = sigmoid(4 * sum_n mapped * color_n)

Design (8 NeuronCores, full inputs in / full output out):
  * Spatial-shard the canvas: core c renders rows [32c, 32c+32).
  * SUPERTILES: 24 per core, each 32 rows x 16 cols = two vertically
    stacked 16x16 SUB-TILES (A rows 0:16, B rows 16:32).  Exact SAT
    culling (tile rect vs the preimage of the |t|_1<=2 diamond) keeps
    every sub-tile's candidate count <= 32 (max is exactly 32).
  * ONE matmul per supertile computes u=t0+t1, v=t0-t1 for BOTH
    sub-tiles: the contraction carries separate yA/yB coordinate rows
    (K=11, fp16 hi/lo split), stationary [11, 128] = [uA|uB|vA|vB] x
    32 pts, so a single moving column serves a pixel of A AND the
    same-offset pixel of B -> psum [128, 256].  This halves both the
    PE streaming and the downstream elementwise traffic.  The two
    supertiles of a QUAD share one 1-bank [128, 512] PSUM tile.
  * |.| via one ACT Abs per quad (ACT is the only engine that can do a
    full-rate PSUM pass; DVE may read at most one PSUM operand and its
    two SB inputs must share a base partition).  A base-shifting copy
    (out base partition is unconstrained) realigns |vA|,|vB| onto rows
    0:64 - on the DVE for the first half of the quads (lowest latency)
    and on the DMA queues for the rest (the endgame cadence is
    DVE-bound); two DVE max ops then produce d = max(|u|,|v|) for all
    four sub-tiles, written into the quad's [128, 256] d-tile.
  * mapped'' = min(d,2)-2 per quad in one DVE tensor_scalar; colors
    are pre-scaled by -c/2.
  * canvas: one matmul per quad, block-diagonal stationary [128, 32]
    (4 sub-tiles x 3 channels, zero-padded to init the whole PSUM
    slot) -> [32, 256] written into slot q%3 / column block q//3 of a
    single persistent [128, 1024] PSUM canvas region.  One sigmoid per
    canvas BANK (after all six quads writing it - reading a bank while
    later matmuls still target it races on hardware), fp16 output
    DMAs on the sync/gpsimd queues.
  * Canvas matmuls trail the uv stream by 5 quads and input DMAs are
    split per-engine-queue with tiny leading chunks, keeping the PE
    (the bottleneck engine, ~93% utilized) fed end to end.
"""

import math
import os
import sys

import numpy as np

for _p in ("/opt/trn_rl_repo",):
    if _p not in sys.path and os.path.isdir(_p):
        sys.path.insert(0, _p)

# Geometry (matches the reference module's fixed canvas).
H, W = 256, 384
N_CORES = 8
ROWS_PER_CORE = H // N_CORES            # 32
SUB_R, SUB_C = 16, 16                   # sub-tile = 256 px
F_SUB = SUB_R * SUB_C                   # 256 moving columns
N_ST = W // SUB_C                       # 24 supertiles per core
N_QUAD = N_ST // 2                      # 12 quads
CAP = 32                                # points per sub-tile
K11 = 11                                # contraction rows (fp16 hi/lo)
WIDTH_TO_HEIGHT = 384.0 / 256.0

# Set BASS_KERNEL_TRACE=1 to capture an NTFF profile; results land here.
last_run_info = {}


def _hi_lo(x):
    """Split float64 array into fp16 hi + fp16 lo with tiny residual."""
    hi = x.astype(np.float16)
    lo = (x - hi.astype(np.float64)).astype(np.float16)
    return hi, lo


def _prepare(locations, matrix_offsets, matrix_scale_exponents, colors):
    """Host-side prep: per-point affine combos, SAT culling, packing."""
    loc = np.asarray(locations, np.float64).reshape(-1, 2)      # (N, 2) y,x
    mo = np.asarray(matrix_offsets, np.float64)                  # (N, 2, 2)
    mse = np.asarray(matrix_scale_exponents, np.float64).reshape(-1)
    cols = np.asarray(colors, np.float64).reshape(-1, 3)         # (N, 3)
    n = loc.shape[0]

    scale = (math.sqrt(n) / 2.0) / np.exp(mse)
    mats = mo + np.eye(2)[None, :, :] * scale[:, None, None]     # (N, 2, 2)
    b = loc[:, 0, None] * mats[:, 0, :] + loc[:, 1, None] * mats[:, 1, :]

    wy_u = mats[:, 0, 0] + mats[:, 0, 1]
    wx_u = mats[:, 1, 0] + mats[:, 1, 1]
    c_u = -(b[:, 0] + b[:, 1])
    wy_v = mats[:, 0, 0] - mats[:, 0, 1]
    wx_v = mats[:, 1, 0] - mats[:, 1, 1]
    c_v = -(b[:, 0] - b[:, 1])

    # Exact SAT cull: tile rect intersects {|u|<=2, |v|<=2} iff all four
    # separating-axis interval tests pass (y, x, u, v axes).
    det = wy_u * wx_v - wx_u * wy_v
    A00 = wx_v / det
    A01 = -wx_u / det
    A10 = -wy_v / det
    A11 = wy_u / det
    y0 = A00 * (-c_u) + A01 * (-c_v)
    x0 = A10 * (-c_u) + A11 * (-c_v)
    hy = 2 * (np.abs(A00) + np.abs(A01))
    hx = 2 * (np.abs(A10) + np.abs(A11))

    ys = np.linspace(-1.0, 1.0, H).astype(np.float32).astype(np.float64)
    xs = np.linspace(-WIDTH_TO_HEIGHT, WIDTH_TO_HEIGHT, W).astype(np.float32).astype(np.float64)
    gyh, gyl = _hi_lo(ys)
    gxh, gxl = _hi_lo(xs)

    wyu_h, wyu_l = _hi_lo(wy_u)
    wxu_h, wxu_l = _hi_lo(wx_u)
    cu_h, cu_l = _hi_lo(c_u)
    wyv_h, wyv_l = _hi_lo(wy_v)
    wxv_h, wxv_l = _hi_lo(wx_v)
    cv_h, cv_l = _hi_lo(c_v)

    w_np = np.zeros((N_CORES, K11, N_ST * 128), np.float16)
    g_np = np.zeros((N_CORES, K11, N_ST * F_SUB), np.float16)
    # Canvas stationary padded to 32 cols per quad (cols 12:32 zero) so
    # each matmul initializes a full 32-partition PSUM slot.
    ct_np = np.zeros((N_CORES, 128, N_QUAD * 32), np.float16)

    def cull(r0, c0, rr, cc):
        ylo, yhi = ys[r0], ys[r0 + rr - 1]
        xlo, xhi = xs[c0], xs[c0 + cc - 1]
        yc, xc = (ylo + yhi) / 2, (xlo + xhi) / 2
        ry, rx = (yhi - ylo) / 2, (xhi - xlo) / 2
        ok = (np.abs(yc - y0) <= ry + hy + 1e-9) & \
             (np.abs(xc - x0) <= rx + hx + 1e-9)
        uc = wy_u * yc + wx_u * xc + c_u
        du = np.abs(wy_u) * ry + np.abs(wx_u) * rx
        ok &= np.abs(uc) <= 2 + du + 1e-9
        vc = wy_v * yc + wx_v * xc + c_v
        dv = np.abs(wy_v) * ry + np.abs(wx_v) * rx
        ok &= np.abs(vc) <= 2 + dv + 1e-9
        return np.nonzero(ok)[0]

    for core in range(N_CORES):
        for st in range(N_ST):
            c0 = st * SUB_C
            r0 = core * ROWS_PER_CORE
            wslab = w_np[core, :, 128 * st:128 * (st + 1)]
            for half in range(2):
                rh = r0 + half * SUB_R
                idx = cull(rh, c0, SUB_R, SUB_C)
                m = len(idx)
                assert m <= CAP, f"sub-tile candidate overflow: {m} > {CAP}"
                # Stationary cols: uA 0:32 | uB 32:64 | vA 64:96 | vB 96:128
                # Rows: 0-2 yA(h,l,h) | 3-5 yB | 6-8 x(h,l,h) | 9-10 c(h,l)
                yr = 3 * half
                for voff, (wyh_, wyl_, wxh_, wxl_, ch_, cl_) in (
                        (0, (wyu_h, wyu_l, wxu_h, wxu_l, cu_h, cu_l)),
                        (64, (wyv_h, wyv_l, wxv_h, wxv_l, cv_h, cv_l))):
                    o = voff + 32 * half
                    if m:
                        wslab[yr + 0, o:o + m] = wyh_[idx]
                        wslab[yr + 1, o:o + m] = wyh_[idx]
                        wslab[yr + 2, o:o + m] = wyl_[idx]
                        wslab[6, o:o + m] = wxh_[idx]
                        wslab[7, o:o + m] = wxh_[idx]
                        wslab[8, o:o + m] = wxl_[idx]
                        wslab[9, o:o + m] = ch_[idx]
                        wslab[10, o:o + m] = cl_[idx]
                # mapped'' = min(d,2)-2 = -2*mapped -> colors * -c/2.
                # Quad q = st//2; quarter u = 2*(st%2) + half.
                q, sthalf = divmod(st, 2)
                u = 2 * sthalf + half
                if m:
                    ct_np[core, 32 * u:32 * u + m,
                          32 * q + 3 * u:32 * q + 3 * u + 3] = (
                        -0.5 * cols[idx]).astype(np.float16)

            # Moving G [K11, F_SUB]: px = r*SUB_C + c (row-major in sub).
            go = st * F_SUB
            for half in range(2):
                rh = r0 + half * SUB_R
                yr = 3 * half
                g_np[core, yr + 0, go:go + F_SUB] = np.repeat(gyh[rh:rh + SUB_R], SUB_C)
                g_np[core, yr + 1, go:go + F_SUB] = np.repeat(gyl[rh:rh + SUB_R], SUB_C)
                g_np[core, yr + 2, go:go + F_SUB] = g_np[core, yr + 0, go:go + F_SUB]
            g_np[core, 6, go:go + F_SUB] = np.tile(gxh[c0:c0 + SUB_C], SUB_R)
            g_np[core, 7, go:go + F_SUB] = np.tile(gxl[c0:c0 + SUB_C], SUB_R)
            g_np[core, 8, go:go + F_SUB] = g_np[core, 6, go:go + F_SUB]
            g_np[core, 9, go:go + F_SUB] = 1.0
            g_np[core, 10, go:go + F_SUB] = 1.0

    return w_np, g_np, ct_np


def _build_nc():
    """Build the Bass/Tile program (shared by all cores)."""
    from contextlib import ExitStack

    import concourse.bacc as bacc
    import concourse.tile as tile
    from concourse import mybir

    f16 = mybir.dt.float16
    f32 = mybir.dt.float32
    nc = bacc.Bacc("TRN2", target_bir_lowering=False, debug=False,
                   num_devices=N_CORES)

    w_d = nc.dram_tensor("w", [K11, N_ST * 128], f16, kind="ExternalInput")
    g_d = nc.dram_tensor("g", [K11, N_ST * F_SUB], f16, kind="ExternalInput")
    ct_d = nc.dram_tensor("ct", [128, N_QUAD * 32], f16, kind="ExternalInput")
    # y[s, p, j, px]: canvas slot s (=q%3), p = 3u+ch, col-block j (=q//3)
    y_d = nc.dram_tensor("y", [3, 12, 4, F_SUB], f16, kind="ExternalOutput")

    with ExitStack() as ctx:
        tc = ctx.enter_context(tile.TileContext(nc))
        const = ctx.enter_context(tc.tile_pool(name="const", bufs=1))
        uvpool = ctx.enter_context(tc.tile_pool(name="uv", bufs=6, space="PSUM"))
        cvpool = ctx.enter_context(tc.tile_pool(name="cv", bufs=1, space="PSUM"))
        apool = ctx.enter_context(tc.tile_pool(name="a", bufs=8))
        spool = ctx.enter_context(tc.tile_pool(name="s", bufs=8))
        mpool = ctx.enter_context(tc.tile_pool(name="m", bufs=8))
        rpool = ctx.enter_context(tc.tile_pool(name="r", bufs=8))
        opool = ctx.enter_context(tc.tile_pool(name="o", bufs=1))

        W_sb = const.tile([K11, N_ST * 128], f16)
        G_sb = const.tile([K11, N_ST * F_SUB], f16)
        CT_sb = const.tile([128, N_QUAD * 32], f16)
        # W and CT gate LDWEIGHTS; G is partition-bandwidth bound, so
        # split across the three DMA-capable engine queues with tiny
        # leading chunks so the first matmuls start as early as possible.
        # The first W and G chunks lead their queues so the first
        # matmul starts early; scalar's one DMA precedes any Abs work.
        nc.sync.dma_start(W_sb[:, 0:2 * 128], w_d[:, 0:2 * 128])
        nc.scalar.dma_start(G_sb[:, 0:2 * F_SUB], g_d[:, 0:2 * F_SUB])
        nc.gpsimd.dma_start(CT_sb[:], ct_d[:])
        nc.sync.dma_start(W_sb[:, 2 * 128:], w_d[:, 2 * 128:])
        g_chunks = [(2, 4, nc.gpsimd), (4, 8, nc.sync), (8, 12, nc.gpsimd),
                    (12, 16, nc.sync), (16, 20, nc.gpsimd),
                    (20, 24, nc.sync)]
        for lo_t, hi_t, eng in g_chunks:
            eng.dma_start(G_sb[:, lo_t * F_SUB:hi_t * F_SUB],
                          g_d[:, lo_t * F_SUB:hi_t * F_SUB])

        # Pin the act table that holds BOTH Abs and Sigmoid by issuing a
        # tiny sigmoid first; all later activations then share one table.
        warm = opool.tile([128, 1], f32, tag="warm", bufs=1)
        nc.scalar.activation(warm[:], CT_sb[:, 0:1],
                             mybir.ActivationFunctionType.Sigmoid)

        # Single canvas region: quad q -> slot q%3 (partition base), col
        # block q//3.  All 12 quads land before the one sigmoid.
        canvas = cvpool.tile([128, 4 * F_SUB], f32, bufs=1)

        def canvas_mm(q, mr):
            s, j = q % 3, q // 3
            nc.tensor.matmul(canvas[32 * s:32 * s + 32,
                                    F_SUB * j:F_SUB * (j + 1)],
                             CT_sb[:, 32 * q:32 * q + 32], mr[:],
                             start=True, stop=True)

        # Sigmoid + readout by column half so the first half (and its
        # DMAs) overlaps the final quads' compute.
        outr = opool.tile([128, 4 * F_SUB], f16, tag="out", bufs=1)

        def sigmoid_out(hh):
            # One sigmoid per PSUM BANK (cols 512*hh..+512), emitted only
            # after every quad writing that bank has landed: a PSUM bank
            # is the accumulation-reset granule, so reading a bank while
            # later matmuls still target it races on real hardware.
            cl, ch = 2 * F_SUB * hh, 2 * F_SUB * (hh + 1)
            nc.scalar.activation(outr[0:96, cl:ch], canvas[0:96, cl:ch],
                                 mybir.ActivationFunctionType.Sigmoid,
                                 scale=4.0)
            for s, eng in zip(range(3), (nc.sync, nc.gpsimd, nc.sync)):
                eng.dma_start(y_d[s, :, 2 * hh:2 * hh + 2, :],
                              outr[32 * s:32 * s + 12, cl:ch])

        pend = []
        for q in range(N_QUAD):
            # Both supertiles of the quad share one 1-bank PSUM tile.
            puv = uvpool.tile([128, 2 * F_SUB], f32, tag="uv", bufs=6)
            for sthalf in range(2):
                st = 2 * q + sthalf
                nc.tensor.matmul(puv[:, F_SUB * sthalf:F_SUB * (sthalf + 1)],
                                 W_sb[:, 128 * st:128 * (st + 1)],
                                 G_sb[:, F_SUB * st:F_SUB * (st + 1)],
                                 start=True, stop=True)
            # |uA|,|uB|,|vA|,|vB| x2: one PSUM pass per quad (ACT).
            ab = apool.tile([128, 2 * F_SUB], f16, tag="ab")
            nc.scalar.activation(ab[:], puv[:],
                                 mybir.ActivationFunctionType.Abs)
            # DVE needs equal input base partitions for two-input ops,
            # but a single-input copy may shift bases.  Front half: DVE
            # copy (lowest latency while the pipeline fills).  Back
            # half: DMA-queue shifts, because the endgame cadence is
            # DVE-bound (copy+2*max+clamp) once the uv stream drains.
            sh = spool.tile([64, 2 * F_SUB], f16, tag="sh")
            if q < 6 or q >= 10:
                # DVE copy: lowest latency.  Used while the pipeline
                # fills AND for the final quads, whose chains are pure
                # tail latency (a DMA round-trip there directly
                # lengthens the kernel).
                nc.vector.tensor_copy(sh[0:64, :], ab[64:128, :])
            else:
                nc.sync.dma_start(sh[0:64, 0:F_SUB], ab[64:128, 0:F_SUB])
                nc.gpsimd.dma_start(sh[0:64, F_SUB:], ab[64:128, F_SUB:])
            # d for both sub-tiles of each supertile; output base is
            # free, so each supertile fills its half of the quad tile.
            dm = mpool.tile([128, F_SUB], f16, tag="m")
            for sthalf in range(2):
                cl, ch = F_SUB * sthalf, F_SUB * (sthalf + 1)
                nc.vector.tensor_tensor(dm[64 * sthalf:64 * sthalf + 64, :],
                                        ab[0:64, cl:ch], sh[0:64, cl:ch],
                                        op=mybir.AluOpType.max)
            # mapped'' = min(d,2)-2 for the whole quad.
            mr = rpool.tile([128, F_SUB], f16, tag="mr")
            nc.vector.tensor_scalar(
                mr[:], dm[:], 2.0, 2.0,
                op0=mybir.AluOpType.min, op1=mybir.AluOpType.subtract)
            pend.append((q, mr))
            # Delay canvas by four quads: keeps the PE stream dense.
            if len(pend) > 5:
                done = pend.pop(0)
                canvas_mm(*done)
                if done[0] == 5:
                    sigmoid_out(0)   # bank 0 (quads 0-5) complete
        while pend:
            done = pend.pop(0)
            canvas_mm(*done)
            if done[0] == 5:
                sigmoid_out(0)
        sigmoid_out(1)

    nc.compile()
    return nc


def _install_ntff_hook():
    """Provide antenv.axon_hooks if the image lacks it (ctypes shim around
    libaxon_pjrt.so's NRT profile capture). Returns True on success."""
    try:
        from antenv.axon_hooks import get_axon_ntff_profile_hook  # noqa: F401
        return True
    except ImportError:
        pass
    try:
        import contextlib
        import ctypes
        import types

        import antenv

        so_path = "/opt/axon/libaxon_pjrt.so"
        lib = ctypes.CDLL(so_path)
        if not hasattr(lib, "axon_start_nrt_profile"):
            return False
        lib.axon_start_nrt_profile.argtypes = [
            ctypes.POINTER(ctypes.c_int64), ctypes.c_size_t]
        lib.axon_start_nrt_profile.restype = ctypes.c_int64
        lib.axon_stop_nrt_profile.argtypes = [ctypes.c_char_p]
        lib.axon_stop_nrt_profile.restype = ctypes.c_int64

        @contextlib.contextmanager
        def _hook(output_dir, device_ids):
            import jax
            jax.devices()
            if device_ids:
                ids = (ctypes.c_int64 * len(device_ids))(*device_ids)
                rc = lib.axon_start_nrt_profile(ids, len(device_ids))
            else:
                rc = lib.axon_start_nrt_profile(None, 0)
            if rc != 0:
                raise RuntimeError(f"axon_start_nrt_profile rc={rc}")
            try:
                yield
            finally:
                n = lib.axon_stop_nrt_profile(str(output_dir).encode())
                print(f"ntff profile: {n} file(s) -> {output_dir}", file=sys.stderr)

        mod = types.ModuleType("antenv.axon_hooks")
        mod._hook = _hook
        mod.get_axon_ntff_profile_hook = lambda: _hook
        mod.set_axon_ntff_profile_hook = lambda h: None
        sys.modules["antenv.axon_hooks"] = mod
        antenv.axon_hooks = mod
        return True
    except Exception as e:  # pragma: no cover
        print("ntff hook install failed:", e, file=sys.stderr)
        return False


def _unshard(results):
    """Reassemble per-core y [3, 12, 4, 256] into the full (3, H, W)."""
    out = np.empty((3, H, W), np.float32)
    for core in range(N_CORES):
        y = np.asarray(results[core]["y"], np.float32)  # [3, 12, 4, 256]
        for q in range(N_QUAD):
            s, j = q % 3, q // 3
            for u in range(4):
                st = 2 * q + u // 2
                half = u % 2
                blk = y[s, 3 * u:3 * u + 3, j, :]       # [3, 256]
                r0 = core * ROWS_PER_CORE + half * SUB_R
                c0 = st * SUB_C
                out[:, r0:r0 + SUB_R, c0:c0 + SUB_C] = (
                    blk.reshape(3, SUB_R, SUB_C))
    return out


def kernel(locations, matrix_offsets, matrix_scale_exponents, colors,
           canvas_height_px, canvas_width_px):
    assert int(canvas_height_px) == H and int(canvas_width_px) == W

    w_np, g_np, ct_np = _prepare(
        locations, matrix_offsets, matrix_scale_exponents, colors)

    nc = _build_nc()

    from concourse.bass_utils import run_bass_kernel_spmd

    in_maps = [
        {"w": w_np[c], "g": g_np[c], "ct": ct_np[c]} for c in range(N_CORES)
    ]
    trace = bool(int(os.environ.get("BASS_KERNEL_TRACE", "1")))
    if trace:
        trace = _install_ntff_hook()
    try:
        res = run_bass_kernel_spmd(nc, in_maps, core_ids=list(range(N_CORES)),
                                   trace=trace)
    except Exception:
        if not trace:
            raise
        res = run_bass_kernel_spmd(nc, in_maps, core_ids=list(range(N_CORES)),
                                   trace=False)
    last_run_info.clear()
    last_run_info.update(
        exec_time_ns=res.exec_time_ns,
        mean_exec_time_ns=res.mean_exec_time_ns,
        profile_json=res.profile_json,
    )

    return _unshard(res.results)


# revision 94
# speedup vs baseline: 1.1708x; 1.1708x over previous
"""Trainium2 Bass kernel for the nn_Points problem.

Renders N=1024 anisotropic "diamond" points onto a 3x256x384 canvas:
    t = (pixel - loc) @ M_n          (2-vector per pixel per point)
    mapped = relu(1 - (|t0|+|t1|)/2)
    canvas = sigmoid(4 * sum_n mapped * color_n)

Design (8 NeuronCores, full inputs in / full output out):
  * Spatial-shard the canvas: core c renders rows [32c, 32c+32).
  * SUPERTILES: 24 per core, each 32 rows x 16 cols = two vertically
    stacked 16x16 SUB-TILES (A rows 0:16, B rows 16:32).  Exact SAT
    culling (tile rect vs the preimage of the |t|_1<=2 diamond) keeps
    every sub-tile's candidate count <= 32 (max is exactly 32).
  * ONE matmul per supertile computes u=t0+t1, v=t0-t1 for BOTH
    sub-tiles: the contraction carries separate yA/yB coordinate rows
    (K=11, fp16 hi/lo split), stationary [11, 128] = [uA|uB|vA|vB] x
    32 pts, so a single moving column serves a pixel of A AND the
    same-offset pixel of B -> psum [128, 256].  This halves both the
    PE streaming and the downstream elementwise traffic.  The two
    supertiles of a QUAD share one 1-bank [128, 512] PSUM tile.
  * |.| via one ACT Abs per quad (ACT is the only engine that can do a
    full-rate PSUM pass; DVE may read at most one PSUM operand and its
    two SB inputs must share a base partition).  A base-shifting copy
    (out base partition is unconstrained) realigns |vA|,|vB| onto rows
    0:64 - on the DVE for the first half of the quads (lowest latency)
    and on the DMA queues for the rest (the endgame cadence is
    DVE-bound); two DVE max ops then produce d = max(|u|,|v|) for all
    four sub-tiles, written into the quad's [128, 256] d-tile.
  * mapped'' = min(d,2)-2 per quad in one DVE tensor_scalar; colors
    are pre-scaled by -c/2.
  * canvas: one matmul per quad, block-diagonal stationary [128, 32]
    (4 sub-tiles x 3 channels, zero-padded to init the whole PSUM
    slot) -> [32, 256] written into slot q%3 / column block q//3 of a
    single persistent [128, 1024] PSUM canvas region.  One sigmoid per
    canvas BANK (after all six quads writing it - reading a bank while
    later matmuls still target it races on hardware), fp16 output
    DMAs on the sync/gpsimd queues.
  * Canvas matmuls trail the uv stream by 5 quads and input DMAs are
    split per-engine-queue with tiny leading chunks, keeping the PE
    (the bottleneck engine, ~93% utilized) fed end to end.
"""

import math
import os
import sys

import numpy as np

for _p in ("/opt/trn_rl_repo",):
    if _p not in sys.path and os.path.isdir(_p):
        sys.path.insert(0, _p)

# Geometry (matches the reference module's fixed canvas).
H, W = 256, 384
N_CORES = 8
ROWS_PER_CORE = H // N_CORES            # 32
SUB_R, SUB_C = 16, 16                   # sub-tile = 256 px
F_SUB = SUB_R * SUB_C                   # 256 moving columns
N_ST = W // SUB_C                       # 24 supertiles per core
N_QUAD = N_ST // 2                      # 12 quads
CAP = 32                                # points per sub-tile
K11 = 11                                # contraction rows (fp16 hi/lo)
WIDTH_TO_HEIGHT = 384.0 / 256.0

# Set BASS_KERNEL_TRACE=1 to capture an NTFF profile; results land here.
last_run_info = {}


def _hi_lo(x):
    """Split float64 array into fp16 hi + fp16 lo with tiny residual."""
    hi = x.astype(np.float16)
    lo = (x - hi.astype(np.float64)).astype(np.float16)
    return hi, lo


def _prepare(locations, matrix_offsets, matrix_scale_exponents, colors):
    """Host-side prep: per-point affine combos, SAT culling, packing."""
    loc = np.asarray(locations, np.float64).reshape(-1, 2)      # (N, 2) y,x
    mo = np.asarray(matrix_offsets, np.float64)                  # (N, 2, 2)
    mse = np.asarray(matrix_scale_exponents, np.float64).reshape(-1)
    cols = np.asarray(colors, np.float64).reshape(-1, 3)         # (N, 3)
    n = loc.shape[0]

    scale = (math.sqrt(n) / 2.0) / np.exp(mse)
    mats = mo + np.eye(2)[None, :, :] * scale[:, None, None]     # (N, 2, 2)
    b = loc[:, 0, None] * mats[:, 0, :] + loc[:, 1, None] * mats[:, 1, :]

    wy_u = mats[:, 0, 0] + mats[:, 0, 1]
    wx_u = mats[:, 1, 0] + mats[:, 1, 1]
    c_u = -(b[:, 0] + b[:, 1])
    wy_v = mats[:, 0, 0] - mats[:, 0, 1]
    wx_v = mats[:, 1, 0] - mats[:, 1, 1]
    c_v = -(b[:, 0] - b[:, 1])

    # Exact SAT cull: tile rect intersects {|u|<=2, |v|<=2} iff all four
    # separating-axis interval tests pass (y, x, u, v axes).
    det = wy_u * wx_v - wx_u * wy_v
    A00 = wx_v / det
    A01 = -wx_u / det
    A10 = -wy_v / det
    A11 = wy_u / det
    y0 = A00 * (-c_u) + A01 * (-c_v)
    x0 = A10 * (-c_u) + A11 * (-c_v)
    hy = 2 * (np.abs(A00) + np.abs(A01))
    hx = 2 * (np.abs(A10) + np.abs(A11))

    ys = np.linspace(-1.0, 1.0, H).astype(np.float32).astype(np.float64)
    xs = np.linspace(-WIDTH_TO_HEIGHT, WIDTH_TO_HEIGHT, W).astype(np.float32).astype(np.float64)
    gyh, gyl = _hi_lo(ys)
    gxh, gxl = _hi_lo(xs)

    wyu_h, wyu_l = _hi_lo(wy_u)
    wxu_h, wxu_l = _hi_lo(wx_u)
    cu_h, cu_l = _hi_lo(c_u)
    wyv_h, wyv_l = _hi_lo(wy_v)
    wxv_h, wxv_l = _hi_lo(wx_v)
    cv_h, cv_l = _hi_lo(c_v)

    w_np = np.zeros((N_CORES, K11, N_ST * 128), np.float16)
    g_np = np.zeros((N_CORES, K11, N_ST * F_SUB), np.float16)
    # Canvas stationary padded to 32 cols per quad (cols 12:32 zero) so
    # each matmul initializes a full 32-partition PSUM slot.
    ct_np = np.zeros((N_CORES, 128, N_QUAD * 32), np.float16)

    def cull(r0, c0, rr, cc):
        ylo, yhi = ys[r0], ys[r0 + rr - 1]
        xlo, xhi = xs[c0], xs[c0 + cc - 1]
        yc, xc = (ylo + yhi) / 2, (xlo + xhi) / 2
        ry, rx = (yhi - ylo) / 2, (xhi - xlo) / 2
        ok = (np.abs(yc - y0) <= ry + hy + 1e-9) & \
             (np.abs(xc - x0) <= rx + hx + 1e-9)
        uc = wy_u * yc + wx_u * xc + c_u
        du = np.abs(wy_u) * ry + np.abs(wx_u) * rx
        ok &= np.abs(uc) <= 2 + du + 1e-9
        vc = wy_v * yc + wx_v * xc + c_v
        dv = np.abs(wy_v) * ry + np.abs(wx_v) * rx
        ok &= np.abs(vc) <= 2 + dv + 1e-9
        return np.nonzero(ok)[0]

    for core in range(N_CORES):
        for st in range(N_ST):
            c0 = st * SUB_C
            r0 = core * ROWS_PER_CORE
            wslab = w_np[core, :, 128 * st:128 * (st + 1)]
            for half in range(2):
                rh = r0 + half * SUB_R
                idx = cull(rh, c0, SUB_R, SUB_C)
                m = len(idx)
                assert m <= CAP, f"sub-tile candidate overflow: {m} > {CAP}"
                # Stationary cols: uA 0:32 | uB 32:64 | vA 64:96 | vB 96:128
                # Rows: 0-2 yA(h,l,h) | 3-5 yB | 6-8 x(h,l,h) | 9-10 c(h,l)
                yr = 3 * half
                for voff, (wyh_, wyl_, wxh_, wxl_, ch_, cl_) in (
                        (0, (wyu_h, wyu_l, wxu_h, wxu_l, cu_h, cu_l)),
                        (64, (wyv_h, wyv_l, wxv_h, wxv_l, cv_h, cv_l))):
                    o = voff + 32 * half
                    if m:
                        wslab[yr + 0, o:o + m] = wyh_[idx]
                        wslab[yr + 1, o:o + m] = wyh_[idx]
                        wslab[yr + 2, o:o + m] = wyl_[idx]
                        wslab[6, o:o + m] = wxh_[idx]
                        wslab[7, o:o + m] = wxh_[idx]
                        wslab[8, o:o + m] = wxl_[idx]
                        wslab[9, o:o + m] = ch_[idx]
                        wslab[10, o:o + m] = cl_[idx]
                # mapped'' = min(d,2)-2 = -2*mapped -> colors * -c/2.
                # Quad q = st//2; quarter u = 2*(st%2) + half.
                q, sthalf = divmod(st, 2)
                u = 2 * sthalf + half
                if m:
                    ct_np[core, 32 * u:32 * u + m,
                          32 * q + 3 * u:32 * q + 3 * u + 3] = (
                        -0.5 * cols[idx]).astype(np.float16)

            # Moving G [K11, F_SUB]: px = r*SUB_C + c (row-major in sub).
            go = st * F_SUB
            for half in range(2):
                rh = r0 + half * SUB_R
                yr = 3 * half
                g_np[core, yr + 0, go:go + F_SUB] = np.repeat(gyh[rh:rh + SUB_R], SUB_C)
                g_np[core, yr + 1, go:go + F_SUB] = np.repeat(gyl[rh:rh + SUB_R], SUB_C)
                g_np[core, yr + 2, go:go + F_SUB] = g_np[core, yr + 0, go:go + F_SUB]
            g_np[core, 6, go:go + F_SUB] = np.tile(gxh[c0:c0 + SUB_C], SUB_R)
            g_np[core, 7, go:go + F_SUB] = np.tile(gxl[c0:c0 + SUB_C], SUB_R)
            g_np[core, 8, go:go + F_SUB] = g_np[core, 6, go:go + F_SUB]
            g_np[core, 9, go:go + F_SUB] = 1.0
            g_np[core, 10, go:go + F_SUB] = 1.0

    return w_np, g_np, ct_np


def _build_nc():
    """Build the Bass/Tile program (shared by all cores)."""
    from contextlib import ExitStack

    import concourse.bacc as bacc
    import concourse.tile as tile
    from concourse import mybir

    f16 = mybir.dt.float16
    f32 = mybir.dt.float32
    nc = bacc.Bacc("TRN2", target_bir_lowering=False, debug=False,
                   num_devices=N_CORES)

    w_d = nc.dram_tensor("w", [K11, N_ST * 128], f16, kind="ExternalInput")
    g_d = nc.dram_tensor("g", [K11, N_ST * F_SUB], f16, kind="ExternalInput")
    ct_d = nc.dram_tensor("ct", [128, N_QUAD * 32], f16, kind="ExternalInput")
    # y[s, p, j, px]: canvas slot s (=q%3), p = 3u+ch, col-block j (=q//3)
    y_d = nc.dram_tensor("y", [3, 12, 4, F_SUB], f16, kind="ExternalOutput")

    with ExitStack() as ctx:
        tc = ctx.enter_context(tile.TileContext(nc))
        const = ctx.enter_context(tc.tile_pool(name="const", bufs=1))
        uvpool = ctx.enter_context(tc.tile_pool(name="uv", bufs=6, space="PSUM"))
        cvpool = ctx.enter_context(tc.tile_pool(name="cv", bufs=1, space="PSUM"))
        apool = ctx.enter_context(tc.tile_pool(name="a", bufs=6))
        spool = ctx.enter_context(tc.tile_pool(name="s", bufs=6))
        mpool = ctx.enter_context(tc.tile_pool(name="m", bufs=6))
        rpool = ctx.enter_context(tc.tile_pool(name="r", bufs=6))
        opool = ctx.enter_context(tc.tile_pool(name="o", bufs=1))

        W_sb = const.tile([K11, N_ST * 128], f16)
        G_sb = const.tile([K11, N_ST * F_SUB], f16)
        CT_sb = const.tile([128, N_QUAD * 32], f16)
        # W and CT gate LDWEIGHTS; G is partition-bandwidth bound, so
        # split across the three DMA-capable engine queues with tiny
        # leading chunks so the first matmuls start as early as possible.
        # The first W and G chunks lead their queues so the first
        # matmul starts early; scalar's one DMA precedes any Abs work.
        nc.sync.dma_start(W_sb[:, 0:2 * 128], w_d[:, 0:2 * 128])
        nc.scalar.dma_start(G_sb[:, 0:2 * F_SUB], g_d[:, 0:2 * F_SUB])
        nc.gpsimd.dma_start(CT_sb[:], ct_d[:])
        nc.sync.dma_start(W_sb[:, 2 * 128:], w_d[:, 2 * 128:])
        g_chunks = [(2, 4, nc.gpsimd), (4, 8, nc.sync), (8, 12, nc.gpsimd),
                    (12, 16, nc.sync), (16, 20, nc.gpsimd),
                    (20, 24, nc.sync)]
        for lo_t, hi_t, eng in g_chunks:
            eng.dma_start(G_sb[:, lo_t * F_SUB:hi_t * F_SUB],
                          g_d[:, lo_t * F_SUB:hi_t * F_SUB])

        # Pin the act table that holds BOTH Abs and Sigmoid by issuing a
        # tiny sigmoid first; all later activations then share one table.
        warm = opool.tile([128, 1], f32, tag="warm", bufs=1)
        nc.scalar.activation(warm[:], CT_sb[:, 0:1],
                             mybir.ActivationFunctionType.Sigmoid)

        # Single canvas region: quad q -> slot q%3 (partition base), col
        # block q//3.  All 12 quads land before the one sigmoid.
        canvas = cvpool.tile([128, 4 * F_SUB], f32, bufs=1)

        def canvas_mm(q, mr):
            s, j = q % 3, q // 3
            nc.tensor.matmul(canvas[32 * s:32 * s + 32,
                                    F_SUB * j:F_SUB * (j + 1)],
                             CT_sb[:, 32 * q:32 * q + 32], mr[:],
                             start=True, stop=True)

        # Sigmoid + readout by column half so the first half (and its
        # DMAs) overlaps the final quads' compute.
        outr = opool.tile([128, 4 * F_SUB], f16, tag="out", bufs=1)

        def sigmoid_out(hh):
            # One sigmoid per PSUM BANK (cols 512*hh..+512), emitted only
            # after every quad writing that bank has landed: a PSUM bank
            # is the accumulation-reset granule, so reading a bank while
            # later matmuls still target it races on real hardware.
            cl, ch = 2 * F_SUB * hh, 2 * F_SUB * (hh + 1)
            nc.scalar.activation(outr[0:96, cl:ch], canvas[0:96, cl:ch],
                                 mybir.ActivationFunctionType.Sigmoid,
                                 scale=4.0)
            for s, eng in zip(range(3), (nc.sync, nc.gpsimd, nc.sync)):
                eng.dma_start(y_d[s, :, 2 * hh:2 * hh + 2, :],
                              outr[32 * s:32 * s + 12, cl:ch])

        pend = []
        for q in range(N_QUAD):
            # Both supertiles of the quad share one 1-bank PSUM tile.
            puv = uvpool.tile([128, 2 * F_SUB], f32, tag="uv", bufs=6)
            for sthalf in range(2):
                st = 2 * q + sthalf
                nc.tensor.matmul(puv[:, F_SUB * sthalf:F_SUB * (sthalf + 1)],
                                 W_sb[:, 128 * st:128 * (st + 1)],
                                 G_sb[:, F_SUB * st:F_SUB * (st + 1)],
                                 start=True, stop=True)
            # |uA|,|uB|,|vA|,|vB| x2: one PSUM pass per quad (ACT).
            ab = apool.tile([128, 2 * F_SUB], f16, tag="ab")
            nc.scalar.activation(ab[:], puv[:],
                                 mybir.ActivationFunctionType.Abs)
            # DVE needs equal input base partitions for two-input ops,
            # but a single-input copy may shift bases.  Front half: DVE
            # copy (lowest latency while the pipeline fills).  Back
            # half: DMA-queue shifts, because the endgame cadence is
            # DVE-bound (copy+2*max+clamp) once the uv stream drains.
            sh = spool.tile([64, 2 * F_SUB], f16, tag="sh")
            if q < 6 or q >= 10:
                # DVE copy: lowest latency.  Used while the pipeline
                # fills AND for the final quads, whose chains are pure
                # tail latency (a DMA round-trip there directly
                # lengthens the kernel).
                nc.vector.tensor_copy(sh[0:64, :], ab[64:128, :])
            else:
                nc.sync.dma_start(sh[0:64, 0:F_SUB], ab[64:128, 0:F_SUB])
                nc.gpsimd.dma_start(sh[0:64, F_SUB:], ab[64:128, F_SUB:])
            # d for both sub-tiles of each supertile; output base is
            # free, so each supertile fills its half of the quad tile.
            dm = mpool.tile([128, F_SUB], f16, tag="m")
            for sthalf in range(2):
                cl, ch = F_SUB * sthalf, F_SUB * (sthalf + 1)
                nc.vector.tensor_tensor(dm[64 * sthalf:64 * sthalf + 64, :],
                                        ab[0:64, cl:ch], sh[0:64, cl:ch],
                                        op=mybir.AluOpType.max)
            # mapped'' = min(d,2)-2 for the whole quad.
            mr = rpool.tile([128, F_SUB], f16, tag="mr")
            nc.vector.tensor_scalar(
                mr[:], dm[:], 2.0, 2.0,
                op0=mybir.AluOpType.min, op1=mybir.AluOpType.subtract)
            pend.append((q, mr))
            # Delay canvas by four quads: keeps the PE stream dense.
            if len(pend) > 5:
                done = pend.pop(0)
                canvas_mm(*done)
                if done[0] == 5:
                    sigmoid_out(0)   # bank 0 (quads 0-5) complete
        while pend:
            done = pend.pop(0)
            canvas_mm(*done)
            if done[0] == 5:
                sigmoid_out(0)
        sigmoid_out(1)

    nc.compile()
    return nc


def _install_ntff_hook():
    """Provide antenv.axon_hooks if the image lacks it (ctypes shim around
    libaxon_pjrt.so's NRT profile capture). Returns True on success."""
    try:
        from antenv.axon_hooks import get_axon_ntff_profile_hook  # noqa: F401
        return True
    except ImportError:
        pass
    try:
        import contextlib
        import ctypes
        import types

        import antenv

        so_path = "/opt/axon/libaxon_pjrt.so"
        lib = ctypes.CDLL(so_path)
        if not hasattr(lib, "axon_start_nrt_profile"):
            return False
        lib.axon_start_nrt_profile.argtypes = [
            ctypes.POINTER(ctypes.c_int64), ctypes.c_size_t]
        lib.axon_start_nrt_profile.restype = ctypes.c_int64
        lib.axon_stop_nrt_profile.argtypes = [ctypes.c_char_p]
        lib.axon_stop_nrt_profile.restype = ctypes.c_int64

        @contextlib.contextmanager
        def _hook(output_dir, device_ids):
            import jax
            jax.devices()
            if device_ids:
                ids = (ctypes.c_int64 * len(device_ids))(*device_ids)
                rc = lib.axon_start_nrt_profile(ids, len(device_ids))
            else:
                rc = lib.axon_start_nrt_profile(None, 0)
            if rc != 0:
                raise RuntimeError(f"axon_start_nrt_profile rc={rc}")
            try:
                yield
            finally:
                n = lib.axon_stop_nrt_profile(str(output_dir).encode())
                print(f"ntff profile: {n} file(s) -> {output_dir}", file=sys.stderr)

        mod = types.ModuleType("antenv.axon_hooks")
        mod._hook = _hook
        mod.get_axon_ntff_profile_hook = lambda: _hook
        mod.set_axon_ntff_profile_hook = lambda h: None
        sys.modules["antenv.axon_hooks"] = mod
        antenv.axon_hooks = mod
        return True
    except Exception as e:  # pragma: no cover
        print("ntff hook install failed:", e, file=sys.stderr)
        return False


def _unshard(results):
    """Reassemble per-core y [3, 12, 4, 256] into the full (3, H, W)."""
    out = np.empty((3, H, W), np.float32)
    for core in range(N_CORES):
        y = np.asarray(results[core]["y"], np.float32)  # [3, 12, 4, 256]
        for q in range(N_QUAD):
            s, j = q % 3, q // 3
            for u in range(4):
                st = 2 * q + u // 2
                half = u % 2
                blk = y[s, 3 * u:3 * u + 3, j, :]       # [3, 256]
                r0 = core * ROWS_PER_CORE + half * SUB_R
                c0 = st * SUB_C
                out[:, r0:r0 + SUB_R, c0:c0 + SUB_C] = (
                    blk.reshape(3, SUB_R, SUB_C))
    return out


def kernel(locations, matrix_offsets, matrix_scale_exponents, colors,
           canvas_height_px, canvas_width_px):
    assert int(canvas_height_px) == H and int(canvas_width_px) == W

    w_np, g_np, ct_np = _prepare(
        locations, matrix_offsets, matrix_scale_exponents, colors)

    nc = _build_nc()

    from concourse.bass_utils import run_bass_kernel_spmd

    in_maps = [
        {"w": w_np[c], "g": g_np[c], "ct": ct_np[c]} for c in range(N_CORES)
    ]
    trace = bool(int(os.environ.get("BASS_KERNEL_TRACE", "1")))
    if trace:
        trace = _install_ntff_hook()
    try:
        res = run_bass_kernel_spmd(nc, in_maps, core_ids=list(range(N_CORES)),
                                   trace=trace)
    except Exception:
        if not trace:
            raise
        res = run_bass_kernel_spmd(nc, in_maps, core_ids=list(range(N_CORES)),
                                   trace=False)
    last_run_info.clear()
    last_run_info.update(
        exec_time_ns=res.exec_time_ns,
        mean_exec_time_ns=res.mean_exec_time_ns,
        profile_json=res.profile_json,
    )

    return _unshard(res.results)
